# revision 3
# baseline (speedup 1.0000x reference)
import sys

if "/opt/trn_rl_repo" not in sys.path:
    sys.path.insert(0, "/opt/trn_rl_repo")

from contextlib import ExitStack

import numpy as np

import concourse.bass as bass
from concourse import mybir
from concourse.bass_utils import run_bass_kernel_spmd

# Problem constants (hardcoded per spec)
WIDTH, HEIGHT = 320, 240
NUM_BONES, NUM_VERTS, BATCH = 16, 48, 16
BG_DEPTH = 100.0
CLAMP_MIN = 0.01
NCORES = 8
B_LOC = BATCH // NCORES          # 2 batches per core
MAPS = B_LOC * NUM_VERTS         # 96 (b, v) maps per core
HC = HEIGHT // 2                 # 120-row chunks (2 per map)
F32 = mybir.dt.float32

_NC_CACHE = {}


def _build_bass():
    """Device program: for each of 96 (b,v) maps, PE computes the outer sum
    t[h,w] = row2[w] - col2[h] (= r^2 - dx^2 - dy^2), ScalarE does
    relu(t - 0.01) and sqrt(u + 0.01), VectorE builds the background mask
    and the final combine, GpSimd min-accumulates the depth map, and HWDGE
    DMAs stream results to HBM.  Manual semaphore pipeline, fully unrolled."""
    if "nc" in _NC_CACHE:
        return _NC_CACHE["nc"]
    nc = bass.Bass()
    AF = mybir.ActivationFunctionType
    OP = mybir.AluOpType

    # activation() wants float biases available as const APs
    for cval in (-CLAMP_MIN, CLAMP_MIN):
        t = nc.alloc_sbuf_tensor(f"constf32_{cval}", [128, 1], F32)
        nc.gpsimd.memset(t.ap(), cval)
        nc.const_aps.aps[(F32, cval)] = t.ap()
    nc.all_engine_barrier()

    # DRAM I/O (per core)
    lt0 = nc.dram_tensor("LT0", [2, NUM_VERTS * HEIGHT], F32, kind="ExternalInput")
    lt1 = nc.dram_tensor("LT1", [2, NUM_VERTS * HEIGHT], F32, kind="ExternalInput")
    ra0 = nc.dram_tensor("RA0", [2, NUM_VERTS * WIDTH], F32, kind="ExternalInput")
    ra1 = nc.dram_tensor("RA1", [2, NUM_VERTS * WIDTH], F32, kind="ExternalInput")
    zb_d = nc.dram_tensor("ZB", [HC, MAPS], F32, kind="ExternalInput")
    cb_d = nc.dram_tensor("CB", [HC, MAPS], F32, kind="ExternalInput")
    part = nc.dram_tensor(
        "part", [B_LOC, NUM_VERTS, HEIGHT, WIDTH], F32, kind="ExternalOutput"
    )
    depth = nc.dram_tensor("depth", [B_LOC, HEIGHT, WIDTH], F32, kind="ExternalOutput")

    NU = 2   # u/s/mc double buffers
    NO = 3   # out triple buffer

    # nth completed out-DMA after finishing map m (2 chunk DMAs per map,
    # plus the 2 depth DMAs inserted after map 47)
    def dma_ord(m):
        return 2 * (m + 1) + (2 if m >= MAPS // 2 else 0)

    with ExitStack() as ctx:
        e = ctx.enter_context
        lt_s = e(nc.sbuf_tensor("lt_s", [2, NUM_VERTS * HEIGHT], F32))
        ra_s = e(nc.sbuf_tensor("ra_s", [2, NUM_VERTS * WIDTH], F32))
        zb = e(nc.sbuf_tensor("zb", [HC, MAPS], F32))
        cb = e(nc.sbuf_tensor("cb", [HC, MAPS], F32))
        u_t = [e(nc.sbuf_tensor(f"u{k}", [HC, 2 * WIDTH], F32)) for k in range(NU)]
        s_t = [e(nc.sbuf_tensor(f"s{k}", [HC, 2 * WIDTH], F32)) for k in range(NU)]
        mc_t = [e(nc.sbuf_tensor(f"mc{k}", [HC, 2 * WIDTH], F32)) for k in range(NU)]
        o_t = [e(nc.sbuf_tensor(f"o{k}", [HC, 2 * WIDTH], F32)) for k in range(NO)]
        dmin = [e(nc.sbuf_tensor(f"dmin{k}", [HC, 2 * WIDTH], F32)) for k in range(B_LOC)]
        ps = [e(nc.psum_tensor(f"ps{k}", [HC, WIDTH], F32)) for k in range(8)]

        sem_load = e(nc.semaphore("sem_load"))
        sem_pe = e(nc.semaphore("sem_pe"))
        sem_sqrt = e(nc.semaphore("sem_sqrt"))
        sem_mc = e(nc.semaphore("sem_mc"))
        sem_stt = e(nc.semaphore("sem_stt"))
        sem_gmin = e(nc.semaphore("sem_gmin"))
        sem_dma = e(nc.semaphore("sem_dma"))

        with nc.Block() as block:

            @block.tensor
            def _(pe):
                for i in range(MAPS):
                    v = i % NUM_VERTS
                    if i == 0:
                        pe.wait_ge(sem_load, 64)
                    if i == MAPS // 2:
                        pe.wait_ge(sem_load, 96)
                    if i >= 4:
                        pe.wait_ge(sem_sqrt, i - 3)
                    for c in range(2):
                        ins = nc.tensor.matmul(
                            ps[(2 * i + c) % 8][:, :],
                            lt_s[:, v * HEIGHT + c * HC : v * HEIGHT + c * HC + HC],
                            ra_s[:, v * WIDTH : (v + 1) * WIDTH],
                            start=True,
                            stop=True,
                        )
                        if c == 1:
                            ins.then_inc(sem_pe, 1)

            @block.scalar
            def _(sc):
                for i in range(MAPS):
                    sc.wait_ge(sem_pe, i + 1)
                    if i >= NU:
                        sc.wait_ge(sem_mc, i - NU + 1)
                    for c in range(2):
                        nc.scalar.activation(
                            out=u_t[i % NU][:, c * WIDTH : (c + 1) * WIDTH],
                            in_=ps[(2 * i + c) % 8][:, :],
                            func=AF.Relu,
                            bias=-CLAMP_MIN,
                            scale=1.0,
                        )
                    if i >= NU:
                        sc.wait_ge(sem_stt, i - NU + 1)
                    nc.scalar.activation(
                        out=s_t[i % NU][:, :],
                        in_=u_t[i % NU][:, :],
                        func=AF.Sqrt,
                        bias=CLAMP_MIN,
                        scale=1.0,
                    ).then_inc(sem_sqrt, 1)

            @block.vector
            def _(ve):
                for b in range(B_LOC):
                    nc.vector.memset(dmin[b][:, :], 3.0e38)
                ve.wait_ge(sem_load, 64)
                for i in range(MAPS):
                    b = i // NUM_VERTS
                    ve.wait_ge(sem_sqrt, i + 1)
                    nc.vector.tensor_scalar(
                        out=mc_t[i % NU][:, :],
                        in0=u_t[i % NU][:, :],
                        scalar1=0.0,
                        scalar2=cb[:, i : i + 1],
                        op0=OP.is_le,
                        op1=OP.mult,
                    ).then_inc(sem_mc, 1)
                    if i >= NO:
                        ve.wait_ge(sem_dma, 16 * dma_ord(i - NO))
                    nc.vector.scalar_tensor_tensor(
                        out=o_t[i % NO][:, :],
                        in0=mc_t[i % NU][:, :],
                        scalar=zb[:, i : i + 1],
                        in1=s_t[i % NU][:, :],
                        op0=OP.add,
                        op1=OP.subtract,
                    ).then_inc(sem_stt, 1)
                    nc.vector.tensor_tensor(
                        dmin[b][:, :], dmin[b][:, :], o_t[i % NO][:, :], OP.min
                    ).then_inc(sem_gmin, 1)

            @block.sync
            def _(sp):
                for src, dst in ((lt0, lt_s), (ra0, ra_s), (zb_d, zb), (cb_d, cb)):
                    sp.dma_start(out=dst[:, :], in_=src[:, :]).then_inc(sem_load, 16)
                for i in range(MAPS):
                    b, v = i // NUM_VERTS, i % NUM_VERTS
                    sp.wait_ge(sem_stt, i + 1)
                    for c in range(2):
                        sp.dma_start(
                            out=part[b, v, c * HC : (c + 1) * HC, :],
                            in_=o_t[i % NO][:, c * WIDTH : (c + 1) * WIDTH],
                        ).then_inc(sem_dma, 16)
                    if i == MAPS // 2 - 1:
                        sp.wait_ge(sem_gmin, MAPS // 2)
                        for c in range(2):
                            sp.dma_start(
                                out=depth[0, c * HC : (c + 1) * HC, :],
                                in_=dmin[0][:, c * WIDTH : (c + 1) * WIDTH],
                            ).then_inc(sem_dma, 16)
                        sp.wait_ge(sem_pe, MAPS // 2)
                        sp.dma_start(out=lt_s[:, :], in_=lt1[:, :]).then_inc(sem_load, 16)
                        sp.dma_start(out=ra_s[:, :], in_=ra1[:, :]).then_inc(sem_load, 16)
                sp.wait_ge(sem_gmin, MAPS)
                for c in range(2):
                    sp.dma_start(
                        out=depth[1, c * HC : (c + 1) * HC, :],
                        in_=dmin[1][:, c * WIDTH : (c + 1) * WIDTH],
                    ).then_inc(sem_dma, 16)
                sp.wait_ge(sem_dma, 16 * (2 * MAPS + 4))

    _NC_CACHE["nc"] = nc
    return nc


def _host_prep(transformation_mats, vertices, radiuses, bone_idx):
    """Per-core input shards: skinning + per-map row/column squared-distance
    vectors, laid out as the matmul operands the device program expects."""
    f32 = np.float32
    tm = np.asarray(transformation_mats, f32)
    verts = np.asarray(vertices, f32)
    rad = np.asarray(radiuses, f32)
    bidx = np.asarray(bone_idx)

    T = tm[:, bidx]                                  # [B, V, 4, 4]
    skinned = np.einsum("bvij,vj->bvi", T, verts)    # [B, V, 4] (f32)
    x, y, z = skinned[..., 0], skinned[..., 1], skinned[..., 2]

    xg = (np.arange(WIDTH, dtype=f32) - f32(WIDTH / 2)) * f32(300.0 / WIDTH)
    yg = (np.arange(HEIGHT, dtype=f32) - f32(HEIGHT / 2)) * f32(300.0 / HEIGHT)
    r2 = rad * rad                                   # [V]

    dx2 = (xg[None, None, :] - x[..., None]) ** 2    # [B, V, W]
    dy2 = (yg[None, None, :] - y[..., None]) ** 2    # [B, V, H]
    row2 = r2[None, :, None] - dx2                   # [B, V, W]
    col2 = dy2                                       # [B, V, H]

    c100 = f32(BG_DEPTH) + f32(np.sqrt(f32(CLAMP_MIN)))

    in_maps = []
    for m in range(NCORES):
        shard = {}
        for lb in range(B_LOC):
            bg = B_LOC * m + lb
            lt = np.empty((2, NUM_VERTS * HEIGHT), f32)
            lt[0] = 1.0
            lt[1] = col2[bg].reshape(-1)
            ra = np.empty((2, NUM_VERTS * WIDTH), f32)
            ra[0] = row2[bg].reshape(-1)
            ra[1] = -1.0
            shard[f"LT{lb}"] = lt
            shard[f"RA{lb}"] = ra
        z_all = np.concatenate([z[B_LOC * m + lb] for lb in range(B_LOC)]).astype(f32)
        shard["ZB"] = np.broadcast_to(z_all[None, :], (HC, MAPS)).copy()
        shard["CB"] = np.broadcast_to((c100 - z_all)[None, :], (HC, MAPS)).copy()
        in_maps.append(shard)
    return in_maps


def kernel(transformation_mats, vertices, radiuses, bone_idx):
    nc = _build_bass()
    in_maps = _host_prep(transformation_mats, vertices, radiuses, bone_idx)
    res = run_bass_kernel_spmd(nc, in_maps, core_ids=list(range(NCORES)))
    part = np.concatenate([r["part"] for r in res.results], axis=0)
    depth = np.concatenate([r["depth"] for r in res.results], axis=0)
    return part, depth
